# revision 15
# baseline (speedup 1.0000x reference)
"""Trainium2 Bass kernel for nn_CommNetActor.

Network (per sample, 4 agents, all weights shared across agents):
    H0 = sigmoid(O @ enc_w + enc_b)            [B,4,128]
    H1..H3 = relu chain of 128x128 fc layers
    C  = (sum_j H3[:,j] - H3) / 4              (CommNet neighbour mean)
    H4 = [H3 | C] @ cl4_w + cl4_b
    logits = H4.reshape(B,512) @ dec_w + dec_b
    out = softmax(logits)                      [B,16]

Key algebraic fold used here: since C is linear in the H3 agent slices,
the whole tail (neighbour mean + cl4 + dec) collapses into per-agent
readout matrices applied directly to H3:
    logits[b] = sum_a H3[b,a] @ Wz_a + bias'
    Wz_a  = cl4_w[:128] @ D_a + 0.25 * cl4_w[128:] @ (sum_j D_j - D_a)
    bias' = dec_b + cl4_b @ sum_j D_j,      D_a = dec_w[128a:128a+128]
This removes ~35% of the FLOPs, the cross-agent reduction, and the
concat entirely.

Sigmoid is rewritten as tanh so every ScalarE function used (tanh,
relu, exp) lives in one activation-table set:
    sigmoid(x) = 0.5 + 0.5 tanh(x/2)
    H0 := tanh(0.5 x + 0.5 enc_b);  fc1 folded: W1' = 0.5 W1,
    b1' = fc1_b + 0.5 colsum(fc1_w)

Layout: pure data parallelism over 8 cores (8192 samples each). All
activations live transposed in SBUF as [feature(=partition), column],
columns agent-planar per 1024-sample super-tile; sample s of agent a
sits at column (s//512)*2048 + a*512 + s%512. The input is
pre-transposed on the host (no on-device transpose), packed two
samples per column ([128, 2048] per super-tile) so the input DMA uses
all 128 partitions and the K=64 enc matmul runs as two concurrent
row-group-tiled matmuls. Trunk matmuls use float32r (full fp32
storage, 1 cycle/row PE path, measured ~1e-4 rel err end-to-end).
The readout runs activation-stationary (lhsT = H3 chunk in bf16 so
fast-weight-load applies), producing logits in natural [sample, class]
orientation, so softmax is a plain free-dim reduction. ScalarE uses
only {tanh, relu, exp} = one activation-table set (sigmoid was
rewritten as tanh with the affine folded into fc1's weights).
"""

import numpy as np

import concourse.bass as bass
import concourse.mybir as mybir
import concourse.tile as tile
from concourse import bacc
from concourse.bass import ts
from concourse.bass_utils import run_bass_kernel_spmd

# ---- problem constants (hardcoded per the task contract) ----
B = 65536
A = 4
OBS = 64
D = 128
C = 16
NCORES = 8
BLOC = B // NCORES          # samples per core
ST = 1024                   # samples per super-tile
NST = BLOC // ST
COLS = A * ST               # transposed columns per super-tile
NCHUNK = 512                # matmul moving-dim chunk (one f32 PSUM bank)
GROUPS = ST // D            # 128-sample readout chunks per super-tile

F32 = mybir.dt.float32
F32R = mybir.dt.float32r    # full fp32 storage, fast PE path
BF16 = mybir.dt.bfloat16
AFT = mybir.ActivationFunctionType
ALU = mybir.AluOpType

# matmul input dtype for the main trunk: F32R (fast, ~fp32 storage) or
# F32 (4x slower PE, bit-accurate) or BF16.
TRUNK_DT = F32R

_compiled = {}


def _build_bass():
    # Bacc (not plain Bass): its compile() runs generate_event_semaphores /
    # move_matmul_waits_to_ldweights, which legalize multi-wait instructions
    # down to the TRN2 limit of one sync wait per instruction.
    nc = bacc.Bacc()

    # Input packed two-samples-per-column: partitions 0-63 hold features of
    # the first half of each super-tile's samples, 64-127 the second half.
    # Full 128-partition DMA + the enc matmul runs as two concurrent
    # row-group-tiled K=64 matmuls (tile_position (0,0) / (64,0)).
    ot_d = nc.dram_tensor("ot", [2 * OBS, NST, COLS // 2], TRUNK_DT, kind="ExternalInput")
    ew_d = nc.dram_tensor("enc_w", [2 * OBS, D], TRUNK_DT, kind="ExternalInput")
    w1_d = nc.dram_tensor("w1", [D, D], TRUNK_DT, kind="ExternalInput")
    w2_d = nc.dram_tensor("w2", [D, D], TRUNK_DT, kind="ExternalInput")
    w3_d = nc.dram_tensor("w3", [D, D], TRUNK_DT, kind="ExternalInput")
    wz_d = nc.dram_tensor("wz", [D, A * C], BF16, kind="ExternalInput")
    eb_d = nc.dram_tensor("eb", [D, GROUPS * C], F32, kind="ExternalInput")
    b0_d = nc.dram_tensor("b0", [D, 1], F32, kind="ExternalInput")
    b1_d = nc.dram_tensor("b1", [D, 1], F32, kind="ExternalInput")
    b2_d = nc.dram_tensor("b2", [D, 1], F32, kind="ExternalInput")
    b3_d = nc.dram_tensor("b3", [D, 1], F32, kind="ExternalInput")
    out_d = nc.dram_tensor("probs", [BLOC, C], F32, kind="ExternalOutput")

    with tile.TileContext(nc) as tc:
        with (
            tc.tile_pool(name="consts", bufs=1) as cpool,
            tc.tile_pool(name="ot", bufs=3) as opool,
            tc.tile_pool(name="acts", bufs=8) as hpool,
            tc.tile_pool(name="h3p", bufs=6) as h3pool,
            tc.tile_pool(name="soft", bufs=3) as spool,
            tc.tile_pool(name="mm", bufs=4, space="PSUM") as mmpool,
        ):
            ew_t = cpool.tile([2 * OBS, D], TRUNK_DT, name="ew")
            nc.sync.dma_start(ew_t[:], ew_d[:])
            w_t = {}
            for nm, dd in (("w1", w1_d), ("w2", w2_d), ("w3", w3_d)):
                w_t[nm] = cpool.tile([D, D], TRUNK_DT, name=nm)
                nc.sync.dma_start(w_t[nm][:], dd[:])
            wz_t = cpool.tile([D, A * C], BF16, name="wz")
            nc.sync.dma_start(wz_t[:], wz_d[:])
            eb_t = cpool.tile([D, GROUPS * C], F32, name="eb")
            nc.sync.dma_start(eb_t[:], eb_d[:])
            b_t = {}
            for nm, dd in (("b0", b0_d), ("b1", b1_d), ("b2", b2_d), ("b3", b3_d)):
                b_t[nm] = cpool.tile([D, 1], F32, name=nm)
                nc.sync.dma_start(b_t[nm][:], dd[:])

            # Elementwise bias+relu split across ACT/DVE. Per super-tile, ACT
            # already carries enc tanh (4 ops) + exp, so of the 12 fc groups
            # ACT takes 4 on even / 5 on odd super-tiles (balance point 4.4:
            # ACT 1.07us vs DVE 1.19us per 1024-col op). Strictly alternate
            # engines between consecutive groups (FIFO queues: same-engine
            # runs stack queue-wait onto the PSUM slot residency), and put
            # ACT's fc share late in the super-tile since enc occupies ACT
            # at the start.
            FC_ENG = {
                0: {"w1": "vvvv", "w2": "avav", "w3": "vava"},
                1: {"w1": "vvav", "w2": "avav", "w3": "vava"},
            }

            def biasrelu(eng, dst, ps, bias):
                if eng == "a":
                    nc.scalar.activation(dst, ps, AFT.Relu, bias=bias)
                else:
                    nc.vector.tensor_scalar(dst, ps, bias, 0.0, ALU.add, ALU.max)

            def emit_tail(st, h3h):
                # ---- readout: logits[p, g*16+c] for samples g*128+p ----
                # chunk g covers samples g*128..+127 of half g//4 at column
                # offset (g%4)*128 within agent blocks of that half's tile.
                # lg borrows an mm-pool slot (uses 128 of its 1024 cols):
                # all 8 PSUM banks live in one 4-slot pool.
                lgt = mmpool.tile([D, 1024], F32, tag="mm", name="lgt")
                lg = lgt[:, : GROUPS * C]
                for g in range(GROUPS):
                    half = h3h[g // 4]
                    off = (g % 4) * D
                    for a in range(A):
                        nc.tensor.matmul(
                            lg[:, ts(g, C)],
                            half[:, off + a * 512 : off + a * 512 + D],
                            wz_t[:, ts(a, C)],
                            start=(a == 0), stop=(a == A - 1),
                        )

                # ---- softmax over 16 classes per 16-col group ----
                # exp on ACT (PSUM source); SBUF-only tail on Pool (GpSimd),
                # reciprocal on DVE (no Pool impl).
                e = spool.tile([D, GROUPS * C], F32, tag="e")
                nc.scalar.activation(e[:], lg[:], AFT.Exp)
                f = spool.tile([D, GROUPS * C], F32, tag="f")
                nc.gpsimd.tensor_mul(f[:], e[:], eb_t[:])
                s = spool.tile([D, GROUPS], F32, tag="s")
                nc.vector.reduce_sum(
                    s[:], f[:].rearrange("p (g c) -> p g c", c=C),
                    axis=mybir.AxisListType.X,
                )
                r = spool.tile([D, GROUPS], F32, tag="r")
                nc.vector.reciprocal(r[:], s[:])
                p = spool.tile([D, GROUPS * C], F32, tag="p")
                nc.gpsimd.tensor_mul(
                    p[:].rearrange("p (g c) -> p g c", c=C),
                    f[:].rearrange("p (g c) -> p g c", c=C),
                    r[:].unsqueeze(2).broadcast_to([D, GROUPS, C]),
                )

                # ---- store: row st*1024 + g*128 + p ----
                nc.sync.dma_start(
                    out_d[ts(st, ST), :].rearrange("(g p) c -> p g c", p=D),
                    p[:].rearrange("p (g c) -> p g c", c=C),
                )

            pending = None  # (st, h3h) awaiting readout+softmax
            for st in range(NST):
                eng = FC_ENG[st % 2]
                # ---- input: [128, 2048] two-half packed ----
                ot_t = opool.tile([2 * OBS, COLS // 2], TRUNK_DT, tag="ot")
                nc.sync.dma_start(ot_t[:], ot_d[:, st, :])

                # ---- enc: tanh(0.5 x + 0.5 b); ACT engine ----
                # Per-group tiles: group j = sample-half j//2, agents
                # {2(j%2), 2(j%2)+1}. j order (0,2,1,3) keeps consecutive
                # matmuls in different PE row groups (concurrent).
                h0g = [None] * 4
                for j in (0, 2, 1, 3):
                    hh = j // 2
                    base = (j % 2) * 1024
                    ps = mmpool.tile([D, 1024], F32, tag="mm")
                    for k in range(2):
                        nc.tensor.matmul(
                            ps[:, ts(k, NCHUNK)],
                            ew_t[64 * hh : 64 * (hh + 1), :],
                            ot_t[64 * hh : 64 * (hh + 1),
                                 base + k * NCHUNK : base + (k + 1) * NCHUNK],
                            start=True, stop=True,
                        )
                    h0g[j] = hpool.tile([D, 1024], TRUNK_DT, tag="h0", name=f"h0g{j}")
                    nc.scalar.activation(
                        h0g[j][:], ps[:], AFT.Tanh, bias=b_t["b0"][:], scale=0.5,
                    )

                # ---- fc1 / fc2: per-group pipeline ----
                h1g = [None] * 4
                for j in range(4):
                    ps = mmpool.tile([D, 1024], F32, tag="mm")
                    for k in range(2):
                        nc.tensor.matmul(
                            ps[:, ts(k, NCHUNK)], w_t["w1"][:],
                            h0g[j][:, ts(k, NCHUNK)], start=True, stop=True,
                        )
                    h1g[j] = hpool.tile([D, 1024], TRUNK_DT, tag="h1", name=f"h1g{j}")
                    biasrelu(eng["w1"][j], h1g[j][:], ps[:], b_t["b1"][:])

                h2g = [None] * 4
                for j in range(4):
                    ps = mmpool.tile([D, 1024], F32, tag="mm")
                    for k in range(2):
                        nc.tensor.matmul(
                            ps[:, ts(k, NCHUNK)], w_t["w2"][:],
                            h1g[j][:, ts(k, NCHUNK)], start=True, stop=True,
                        )
                    h2g[j] = hpool.tile([D, 1024], TRUNK_DT, tag="h2", name=f"h2g{j}")
                    biasrelu(eng["w2"][j], h2g[j][:], ps[:], b_t["b2"][:])

                # ---- fc3 -> bf16 H3, one tile per sample-half ----
                h3h = [
                    h3pool.tile([D, 2048], BF16, tag="h3", name="h3h0"),
                    h3pool.tile([D, 2048], BF16, tag="h3", name="h3h1"),
                ]
                for j in range(4):
                    ps = mmpool.tile([D, 1024], F32, tag="mm")
                    for k in range(2):
                        nc.tensor.matmul(
                            ps[:, ts(k, NCHUNK)], w_t["w3"][:],
                            h2g[j][:, ts(k, NCHUNK)], start=True, stop=True,
                        )
                    dst = h3h[j // 2][:, (j % 2) * 1024 : (j % 2) * 1024 + 1024]
                    biasrelu(eng["w3"][j], dst, ps[:], b_t["b3"][:])

                # Software pipeline: the tail of the PREVIOUS super-tile is
                # emitted here, after this super-tile's trunk, so its exp /
                # mul / store ops sit behind trunk work in each engine's
                # FIFO instead of head-of-line blocking the next trunk.
                if pending is not None:
                    emit_tail(*pending)
                pending = (st, h3h)

            emit_tail(*pending)

    nc.compile()
    return nc


def _prep_inputs(inputs):
    """Host-side: fused weights + per-core transposed input shards."""
    f64 = lambda x: np.asarray(x, np.float64)
    enc_w, enc_b = f64(inputs["enc_w"]), f64(inputs["enc_b"])
    fc1_w, fc1_b = f64(inputs["fc1_w"]), f64(inputs["fc1_b"])
    fc2_w, fc2_b = f64(inputs["fc2_w"]), f64(inputs["fc2_b"])
    fc3_w, fc3_b = f64(inputs["fc3_w"]), f64(inputs["fc3_b"])
    cl4_w, cl4_b = f64(inputs["cl4_w"]), f64(inputs["cl4_b"])
    dec_w, dec_b = f64(inputs["dec_w"]), f64(inputs["dec_b"])

    A_ = cl4_w[:D]
    Bm = cl4_w[D:]
    Da = dec_w.reshape(A, D, C)
    Dsum = Da.sum(0)
    Wz = np.concatenate(
        [A_ @ Da[a] + 0.25 * (Bm @ (Dsum - Da[a])) for a in range(A)], axis=1
    )  # [128, 64]
    bias_p = dec_b + cl4_b @ Dsum  # [16]

    import ml_dtypes

    common = {
        "enc_w": np.ascontiguousarray(np.vstack([enc_w, enc_w]), np.float32),
        "w1": np.ascontiguousarray(0.5 * fc1_w, np.float32),
        "w2": np.ascontiguousarray(fc2_w, np.float32),
        "w3": np.ascontiguousarray(fc3_w, np.float32),
        "wz": np.ascontiguousarray(Wz).astype(ml_dtypes.bfloat16),
        "eb": np.tile(np.exp(bias_p).astype(np.float32)[None, :], (D, GROUPS)),
        "b0": (0.5 * enc_b).astype(np.float32).reshape(D, 1),
        "b1": (fc1_b + 0.5 * fc1_w.sum(0)).astype(np.float32).reshape(D, 1),
        "b2": fc2_b.astype(np.float32).reshape(D, 1),
        "b3": fc3_b.astype(np.float32).reshape(D, 1),
    }

    O = np.asarray(inputs["O"], np.float32)  # [B, A, OBS]
    in_maps = []
    for c in range(NCORES):
        oc = O[c * BLOC : (c + 1) * BLOC]                  # [BLOC, A, OBS]
        # ot[h*64+f, st, a*512+s'] = O[st*1024 + h*512 + s', a, f]
        x = oc.reshape(NST, 2, ST // 2, A, OBS)
        ot = np.ascontiguousarray(x.transpose(1, 4, 0, 3, 2)).reshape(
            2 * OBS, NST, COLS // 2
        )
        in_maps.append({"ot": ot, **common})
    return in_maps


def kernel(**inputs):
    if "nc" not in _compiled:
        _compiled["nc"] = _build_bass()
    nc = _compiled["nc"]
    in_maps = _prep_inputs(inputs)
    res = run_bass_kernel_spmd(nc, in_maps, core_ids=list(range(NCORES)))
    return np.concatenate([res.results[i]["probs"] for i in range(NCORES)], axis=0)

